# revision 18
# baseline (speedup 1.0000x reference)
"""Gumbel Top-K gate kernel for Trainium2 (8 NeuronCores, SPMD).

Math: mask[b, 0, r, m] = 1 iff z[b, r, m] is among the top-16 of row r, where
  z = mean_h(q_h k_h^T)/sqrt(64) + gumbel(u),  gumbel = -log(-log(u+eps)+eps).
Softmax is strictly monotone per row, so the reference's softmax/top-k mask
equals thresholding z at its 16th-largest value per row (ties included via >=).

Sharding: core c handles batch b = c//2, row half c%2 -> [1024, 2048] slab.
Head-mean folds into one [1024, 512] x [512, 2048] matmul per core (concat
heads along the contraction dim), fp16 weights/moving (1 cyc/row on the PE).

Per 128-row tile (steady state; DVE top-16 scan is the bottleneck at
~6.7us/tile, everything else hides under it):
  - ACT: gumbel = Ln(R) straight into PSUM (R = 1/(-log(u+eps)+eps) shipped
    from host), emitted for tile t+1 ahead of Copy(t) so the ACT FIFO never
    stalls the PE. After the matmuls an ACT Copy evacuates z to SBUF (zc),
    freeing the PSUM slab.
  - PE: f16 matmuls accumulate z = S + gumbel in place. PSUM has_written
    bits are set once per slab by zero bf16 matmuls (start=True); engine
    writes never clear them and no matmul ever issues stop=True, so the bits
    persist across slab reuse and every later start=False matmul accumulates
    onto the ACT-written gumbel.
  - DVE: max8(zc) -> match_replace(-BIG) -> max8(zs): t16 = 16th largest.
  - ACT: mask = Sign(t16 - zc) -> int8 {-1,0,1}; host maps <=0 to 1.0 (exact
    >= compare: fp32 subtraction never gets the sign wrong). Sign for tile t
    is emitted one iteration late so its wait on the DVE scan sits behind
    tile t+1's Ln/Copy in the ACT FIFO.
  - All DMA issues share the sync HWDGE queue, ordered R0, weights, then
    R prefetches/mask stores at iteration end, so the fill-critical bytes
    stream first.
"""

import sys

sys.path.insert(0, "/opt/trn_rl_repo")

import numpy as np

import concourse.bacc as bacc
import concourse.mybir as mybir
import concourse.tile as tile
from concourse import bass_utils

B, H, N, D = 4, 8, 2048, 64
HD = H * D  # 512 contraction dim (heads concatenated)
N_CORES = 8
ROWS = N * B // N_CORES  # 1024 rows per core
P = 128
EPS = 1e-9
NEG_BIG = -3.0e38
F32 = mybir.dt.float32
F16 = mybir.dt.float16
BF16 = mybir.dt.bfloat16
I8 = mybir.dt.int8
BANK = 512  # fp32 PSUM bank width


def _build_body(tc, qT_d, kT_d, r_d, mask_d):
    nc = tc.nc
    n_rtiles = ROWS // P  # 8
    n_c = HD // P  # 4 contraction chunks
    n_b = N // BANK  # 4 psum banks per row tile
    act = mybir.ActivationFunctionType

    with (
        tc.tile_pool(name="consts", bufs=1) as consts,
        tc.tile_pool(name="kqT", bufs=1) as kqT_pool,
        tc.tile_pool(name="s_psum", bufs=1, space="PSUM") as s_psum,
        tc.tile_pool(name="rin", bufs=3) as rin,
        tc.tile_pool(name="zc_pool", bufs=3) as zc_pool,
        tc.tile_pool(name="zs_pool", bufs=2) as zs_pool,
        tc.tile_pool(name="mout", bufs=2) as mout,
        tc.tile_pool(name="small", bufs=4) as small,
    ):
        # bf16 zeros for the has_written-setting dummy matmuls
        zl = consts.tile([1, P], BF16)
        nc.vector.memset(zl, 0.0)
        zr = consts.tile([1, BANK], BF16)
        nc.vector.memset(zr, 0.0)

        r_t = r_d.rearrange("(t p) n -> t p n", p=P)
        mask_t = mask_d.rearrange("(t p) n -> t p n", p=P)
        # R0 first on the sync queue: Ln(0) hides under the weight stream
        rts = [rin.tile([P, N], F32, tag="r", name=f"rt{i}") for i in range(1)]
        nc.sync.dma_start(out=rts[0], in_=r_t[0])

        # d-major loads straight from host-transposed DRAM; no PE transposes.
        # Interleave kT/qT per chunk so tile 0's matmuls start on chunk 0
        # while later chunks stream.
        kT_r = kT_d.rearrange("(c p) m -> c p m", p=P)
        qT_r = qT_d.rearrange("(c p) m -> c p m", p=P)
        kT = [kqT_pool.tile([P, N], F16, tag=f"kT{c}", name=f"kT{c}") for c in range(n_c)]
        qT = [kqT_pool.tile([P, ROWS], F16, tag=f"qT{c}", name=f"qT{c}") for c in range(n_c)]
        for c in range(n_c):
            nc.sync.dma_start(out=kT[c], in_=kT_r[c])
            nc.sync.dma_start(out=qT[c], in_=qT_r[c])

        # two explicit PSUM slabs (4 banks each); set their has_written bits
        # once up front via zero matmuls, then never issue stop=True so the
        # bits survive slab reuse
        S = [s_psum.tile([P, N], F32, tag=f"S{i}", name=f"S{i}") for i in range(2)]
        for i in range(2):
            for m in range(n_b):
                nc.tensor.matmul(
                    S[i][:, m * BANK : (m + 1) * BANK],
                    zl,
                    zr,
                    start=True,
                    stop=False,
                )
        # gumbel(0) = Ln(R0) -> PSUM slab 0
        nc.scalar.activation(S[0], rts[0], act.Ln)

        pending = None  # (zc, m8b, t): Sign emitted one iteration late
        for t in range(n_rtiles):
            St = S[t % 2]

            for c in range(n_c):
                q_slice = qT[c][:, t * P : (t + 1) * P]
                for m in range(n_b):
                    nc.tensor.matmul(
                        St[:, m * BANK : (m + 1) * BANK],
                        q_slice,
                        kT[c][:, m * BANK : (m + 1) * BANK],
                        start=False,
                        stop=False,
                    )

            # next tile's gumbel goes ahead of Copy(t) in the ACT FIFO so a
            # Copy waiting on matmuls can never stall it
            if t + 1 < n_rtiles:
                rts.append(
                    rin.tile([P, N], F32, tag="r", name=f"rt{t + 1}")
                )
                nc.sync.dma_start(out=rts[t + 1], in_=r_t[t + 1])
                nc.scalar.activation(S[(t + 1) % 2], rts[t + 1], act.Ln)

            # evacuate z to SBUF; PSUM slab is free after this
            zc = zc_pool.tile([P, N], F32, tag="zc")
            nc.scalar.activation(zc, St, act.Copy)

            m8a = small.tile([P, 8], F32, tag="m8a")
            nc.vector.max(out=m8a, in_=zc)
            zs = zs_pool.tile([P, N], F32, tag="zs")
            nc.vector.match_replace(
                out=zs, in_to_replace=m8a, in_values=zc, imm_value=NEG_BIG
            )
            m8b = small.tile([P, 8], F32, tag="m8b")
            nc.vector.max(out=m8b, in_=zs)

            if pending is not None:
                _emit_sign(nc, act, mout, mask_t, *pending)
            pending = (zc, m8b, t)

        _emit_sign(nc, act, mout, mask_t, *pending)


def _emit_sign(nc, act, mout, mask_t, zc, m8b, t):
    # Sign(t16 - z): +1 below threshold, 0 tie, -1 above; host maps <=0 -> 1
    mk = mout.tile([P, N], I8, tag="mk")
    nc.scalar.activation(mk, zc, act.Sign, bias=m8b[:, 7:8], scale=-1.0)
    nc.sync.dma_start(out=mask_t[t], in_=mk)


def build_kernel():
    nc = bacc.Bacc(
        "TRN2", target_bir_lowering=False, debug=False, num_devices=N_CORES
    )
    qT = nc.dram_tensor("qT", [HD, ROWS], F16, kind="ExternalInput").ap()
    kT = nc.dram_tensor("kT", [HD, N], F16, kind="ExternalInput").ap()
    r = nc.dram_tensor("r", [ROWS, N], F32, kind="ExternalInput").ap()
    mask = nc.dram_tensor("mask", [ROWS, N], I8, kind="ExternalOutput").ap()
    with tile.TileContext(nc) as tc:
        _build_body(tc, qT, kT, r, mask)
    nc.compile()
    return nc


_NC_CACHE = None
LAST_RESULTS = None


def _get_nc():
    global _NC_CACHE
    if _NC_CACHE is None:
        _NC_CACHE = build_kernel()
    return _NC_CACHE


def make_in_maps(q, k, u):
    q = np.asarray(q, np.float32)
    k = np.asarray(k, np.float32)
    u = np.asarray(u, np.float32)
    # R = 1/(-log(u+eps)+eps): host-side; device recovers the gumbel as
    # Ln(R) = -log(-log(u+eps)+eps) in one ACT pass. fp64 reciprocal keeps
    # the roundtrip error ~1 ulp.
    l1 = -np.log(u + np.float32(EPS))  # fp32, matches reference's inner log
    r_full = (1.0 / (l1.astype(np.float64) + EPS)).astype(np.float32)
    in_maps = []
    kT_by_batch = {}
    for core in range(N_CORES):
        b, half = divmod(core, 2)
        r0 = half * ROWS
        if b not in kT_by_batch:
            # [N, H, D] -> [H*D, N] d-major
            kT_by_batch[b] = np.ascontiguousarray(
                k[b].transpose(1, 0, 2).reshape(N, HD).T.astype(np.float16)
            )
        # 1/64 scale is an exact power-of-two: no extra rounding on top of
        # the fp16 cast
        qT = np.ascontiguousarray(
            (
                q[b, :, r0 : r0 + ROWS, :].transpose(1, 0, 2).reshape(ROWS, HD).T
                * np.float32(1.0 / 64)
            ).astype(np.float16)
        )
        in_maps.append(
            {
                "qT": qT,
                "kT": kT_by_batch[b],
                "r": np.ascontiguousarray(r_full[b, r0 : r0 + ROWS]),
            }
        )
    return in_maps


def kernel(q, k, u):
    global LAST_RESULTS
    in_maps = make_in_maps(q, k, u)
    res = bass_utils.run_bass_kernel_spmd(
        _get_nc(), in_maps, core_ids=list(range(N_CORES))
    )
    LAST_RESULTS = res
    out = np.empty((B, 1, N, N), np.float32)
    for core in range(N_CORES):
        b, half = divmod(core, 2)
        r0 = half * ROWS
        out[b, 0, r0 : r0 + ROWS] = (
            res.results[core]["mask"] <= 0
        ).astype(np.float32)
    return out


# revision 23
# speedup vs baseline: 1.0119x; 1.0119x over previous
"""Gumbel Top-K gate kernel for Trainium2 (8 NeuronCores, SPMD).

Math: mask[b, 0, r, m] = 1 iff z[b, r, m] is among the top-16 of row r, where
  z = mean_h(q_h k_h^T)/sqrt(64) + gumbel(u),  gumbel = -log(-log(u+eps)+eps).
Softmax is strictly monotone per row, so the reference's softmax/top-k mask
equals thresholding z at its 16th-largest value per row (ties included via >=).

Sharding: core c handles batch b = c//2, row half c%2 -> [1024, 2048] slab.
Head-mean folds into one [1024, 512] x [512, 2048] matmul per core (concat
heads along the contraction dim), fp16 weights/moving (1 cyc/row on the PE).

Per 128-row tile (steady state; DVE top-16 scan is the bottleneck at
~6.7us/tile, everything else hides under it):
  - ACT: gumbel = Ln(R) straight into PSUM (R = 1/(-log(u+eps)+eps) shipped
    from host), emitted for tile t+1 ahead of Copy(t) so the ACT FIFO never
    stalls the PE. After the matmuls an ACT Copy evacuates z to SBUF (zc),
    freeing the PSUM slab.
  - PE: f16 matmuls accumulate z = S + gumbel in place. Tiles 0/1 instead
    matmul with start=True (setting the slab's PSUM has_written bits) and
    the DVE adds the gumbel while it is otherwise idle during the fill; no
    matmul ever issues stop=True, so the bits persist across slab reuse and
    every later start=False matmul accumulates onto the ACT-written gumbel
    (engine writes never clear the bits).
  - DVE: max8(zc) -> match_replace(-BIG) -> max8(zs): t16 = 16th largest.
  - ACT: mask = Sign(t16 - zc) -> int8 {-1,0,1}; host maps <=0 to 1.0 (exact
    >= compare: fp32 subtraction never gets the sign wrong). Sign for tile t
    is emitted one iteration late so its wait on the DVE scan sits behind
    tile t+1's Ln/Copy in the ACT FIFO.
  - DMA: kT/R/mask-stores issue on the sync HWDGE queue (R0 first, then
    weights, then per-iteration R prefetches and stores), qT on the scalar
    HWDGE queue, so fill-critical bytes stream first and issue costs are
    paid in parallel.
"""

import sys

sys.path.insert(0, "/opt/trn_rl_repo")

import numpy as np

import concourse.bacc as bacc
import concourse.mybir as mybir
import concourse.tile as tile
from concourse import bass_utils

B, H, N, D = 4, 8, 2048, 64
HD = H * D  # 512 contraction dim (heads concatenated)
N_CORES = 8
ROWS = N * B // N_CORES  # 1024 rows per core
P = 128
EPS = 1e-9
NEG_BIG = -3.0e38
F32 = mybir.dt.float32
F16 = mybir.dt.float16
BF16 = mybir.dt.bfloat16
I8 = mybir.dt.int8
BANK = 512  # fp32 PSUM bank width


def _build_body(tc, qT_d, kT_d, r_d, mask_d):
    nc = tc.nc
    n_rtiles = ROWS // P  # 8
    n_c = HD // P  # 4 contraction chunks
    n_b = N // BANK  # 4 psum banks per row tile
    act = mybir.ActivationFunctionType

    with (
        tc.tile_pool(name="kqT", bufs=1) as kqT_pool,
        tc.tile_pool(name="s_psum", bufs=1, space="PSUM") as s_psum,
        tc.tile_pool(name="rin", bufs=3) as rin,
        tc.tile_pool(name="gum", bufs=2) as gum_pool,
        tc.tile_pool(name="zc_pool", bufs=3) as zc_pool,
        tc.tile_pool(name="zs_pool", bufs=2) as zs_pool,
        tc.tile_pool(name="mout", bufs=2) as mout,
        tc.tile_pool(name="small", bufs=4) as small,
    ):
        r_t = r_d.rearrange("(t p) n -> t p n", p=P)
        mask_t = mask_d.rearrange("(t p) n -> t p n", p=P)
        # R0 first on the sync queue: Ln(0) hides under the weight stream
        rts = [rin.tile([P, N], F32, tag="r", name=f"rt{i}") for i in range(1)]
        nc.sync.dma_start(out=rts[0], in_=r_t[0])

        # d-major loads straight from host-transposed DRAM; no PE transposes.
        # kT issues on the sync HWDGE queue, qT on the scalar HWDGE queue so
        # the per-issue cost (~0.6us) is paid in parallel during the fill.
        kT_r = kT_d.rearrange("(c p) m -> c p m", p=P)
        qT_r = qT_d.rearrange("(c p) m -> c p m", p=P)
        kT = [kqT_pool.tile([P, N], F16, tag=f"kT{c}", name=f"kT{c}") for c in range(n_c)]
        qT = [kqT_pool.tile([P, ROWS], F16, tag=f"qT{c}", name=f"qT{c}") for c in range(n_c)]
        for c in range(n_c):
            nc.sync.dma_start(out=kT[c], in_=kT_r[c])
            nc.scalar.dma_start(out=qT[c], in_=qT_r[c])

        # two explicit PSUM slabs (4 banks each). Tiles 0/1 matmul with
        # start=True (setting has_written for the whole slab) and add the
        # gumbel afterwards on the DVE; every later tile relies on the bits
        # persisting (no matmul ever issues stop=True) so its start=False
        # matmuls accumulate onto the ACT-written gumbel.
        S = [s_psum.tile([P, N], F32, tag=f"S{i}", name=f"S{i}") for i in range(2)]
        # gumbel(0) = Ln(R0) -> SBUF (tile 0 takes the tensor_add path)
        g0 = gum_pool.tile([P, N], F32, tag="g", name="g0")
        nc.scalar.activation(g0, rts[0], act.Ln)
        gums = [g0]

        pending = None  # (zc, m8b, t): Sign emitted one iteration late
        for t in range(n_rtiles):
            St = S[t % 2]

            for c in range(n_c):
                q_slice = qT[c][:, t * P : (t + 1) * P]
                for m in range(n_b):
                    nc.tensor.matmul(
                        St[:, m * BANK : (m + 1) * BANK],
                        q_slice,
                        kT[c][:, m * BANK : (m + 1) * BANK],
                        start=(t < 2 and c == 0),
                        stop=False,
                    )

            # next tile's gumbel goes ahead of Copy(t) in the ACT FIFO so a
            # Copy waiting on matmuls can never stall it. Tile 1 gets the
            # SBUF/tensor_add treatment like tile 0; later tiles write the
            # gumbel straight into PSUM for the matmuls to accumulate onto.
            if t + 1 < n_rtiles:
                rts.append(
                    rin.tile([P, N], F32, tag="r", name=f"rt{t + 1}")
                )
                nc.sync.dma_start(out=rts[t + 1], in_=r_t[t + 1])
                if t + 1 < 2:
                    g1 = gum_pool.tile([P, N], F32, tag="g", name="g1")
                    nc.scalar.activation(g1, rts[t + 1], act.Ln)
                    gums.append(g1)
                else:
                    nc.scalar.activation(S[(t + 1) % 2], rts[t + 1], act.Ln)

            zc = zc_pool.tile([P, N], F32, tag="zc")
            if t < 2:
                # z = S + gumbel on the DVE (idle during the fill anyway)
                nc.vector.tensor_add(zc, St, gums[t])
            else:
                # evacuate z to SBUF; PSUM slab is free after this
                nc.scalar.activation(zc, St, act.Copy)

            m8a = small.tile([P, 8], F32, tag="m8a")
            nc.vector.max(out=m8a, in_=zc)
            zs = zs_pool.tile([P, N], F32, tag="zs")
            nc.vector.match_replace(
                out=zs, in_to_replace=m8a, in_values=zc, imm_value=NEG_BIG
            )
            m8b = small.tile([P, 8], F32, tag="m8b")
            nc.vector.max(out=m8b, in_=zs)

            if pending is not None:
                _emit_sign(nc, act, mout, mask_t, *pending)
            pending = (zc, m8b, t)

        _emit_sign(nc, act, mout, mask_t, *pending)


def _emit_sign(nc, act, mout, mask_t, zc, m8b, t):
    # Sign(t16 - z): +1 below threshold, 0 tie, -1 above; host maps <=0 -> 1
    mk = mout.tile([P, N], I8, tag="mk")
    nc.scalar.activation(mk, zc, act.Sign, bias=m8b[:, 7:8], scale=-1.0)
    nc.sync.dma_start(out=mask_t[t], in_=mk)


def build_kernel():
    nc = bacc.Bacc(
        "TRN2", target_bir_lowering=False, debug=False, num_devices=N_CORES
    )
    qT = nc.dram_tensor("qT", [HD, ROWS], F16, kind="ExternalInput").ap()
    kT = nc.dram_tensor("kT", [HD, N], F16, kind="ExternalInput").ap()
    r = nc.dram_tensor("r", [ROWS, N], F32, kind="ExternalInput").ap()
    mask = nc.dram_tensor("mask", [ROWS, N], I8, kind="ExternalOutput").ap()
    with tile.TileContext(nc) as tc:
        _build_body(tc, qT, kT, r, mask)
    nc.compile()
    return nc


_NC_CACHE = None
LAST_RESULTS = None


def _get_nc():
    global _NC_CACHE
    if _NC_CACHE is None:
        _NC_CACHE = build_kernel()
    return _NC_CACHE


def make_in_maps(q, k, u):
    q = np.asarray(q, np.float32)
    k = np.asarray(k, np.float32)
    u = np.asarray(u, np.float32)
    # R = 1/(-log(u+eps)+eps): host-side; device recovers the gumbel as
    # Ln(R) = -log(-log(u+eps)+eps) in one ACT pass. fp64 reciprocal keeps
    # the roundtrip error ~1 ulp.
    l1 = -np.log(u + np.float32(EPS))  # fp32, matches reference's inner log
    r_full = (1.0 / (l1.astype(np.float64) + EPS)).astype(np.float32)
    in_maps = []
    kT_by_batch = {}
    for core in range(N_CORES):
        b, half = divmod(core, 2)
        r0 = half * ROWS
        if b not in kT_by_batch:
            # [N, H, D] -> [H*D, N] d-major
            kT_by_batch[b] = np.ascontiguousarray(
                k[b].transpose(1, 0, 2).reshape(N, HD).T.astype(np.float16)
            )
        # 1/64 scale is an exact power-of-two: no extra rounding on top of
        # the fp16 cast
        qT = np.ascontiguousarray(
            (
                q[b, :, r0 : r0 + ROWS, :].transpose(1, 0, 2).reshape(ROWS, HD).T
                * np.float32(1.0 / 64)
            ).astype(np.float16)
        )
        in_maps.append(
            {
                "qT": qT,
                "kT": kT_by_batch[b],
                "r": np.ascontiguousarray(r_full[b, r0 : r0 + ROWS]),
            }
        )
    return in_maps


def kernel(q, k, u):
    global LAST_RESULTS
    in_maps = make_in_maps(q, k, u)
    res = bass_utils.run_bass_kernel_spmd(
        _get_nc(), in_maps, core_ids=list(range(N_CORES))
    )
    LAST_RESULTS = res
    out = np.empty((B, 1, N, N), np.float32)
    for core in range(N_CORES):
        b, half = divmod(core, 2)
        r0 = half * ROWS
        out[b, 0, r0 : r0 + ROWS] = (
            res.results[core]["mask"] <= 0
        ).astype(np.float32)
    return out
